# revision 24
# baseline (speedup 1.0000x reference)
"""Trainium2 Bass kernel for nn_DiffTime (embedding_lookup, 8 NeuronCores).

Computation (see reference):
    h1 = tanh(times * h1_k + h1_b)            [B, 100]
    tv = tanh(h1 @ h2_k + h2_b)               [B, 100]
    mat_x = (emb_x @ evoke_k + evoke_b)       [B, 100p, 100h]   (x in {target, context})
    mv_x = einsum('bph,bh->bp', mat_x, tv)    [B, 100]
    vect_x = mv_x @ last_k + last_b           [B, 300]
    logits = sum(vect_t * vect_c, -1)         [B]
    out = mean(softplus(logits) - logits * labels)

Strategy (data-parallel over batch, 2048 items/core, no collectives):

* tv[b,:] lies on a 1-D curve in R^100; its SVD collapses to rank R=3
  (loss rel err ~8e-7 incl. bf16).  The kernel contracts emb with
  Wr[e,(p,k)] = sum_h evoke[e,p*100+h]*Vr[h,k] and forms
  mv[b,p] = sum_k matU[b,p,k]*cb[b,k] on the DVE (stride-0 broadcast of
  cb, grouped reduce), in bf16.

* Gram fold: for the target branch the kernel uses
  WG[e,(q,k)] = sum_p Wr[e,(p,k)] * Gh[p,q], Gh = lastkh @ lastkh.T
  (host precompute, weights only), so the branch directly yields
  mg[b,q] = sum_p mvt_h[b,p] Gh[p,q] (after adding the constant row
  Gh[100,:]).  logits[b] = sum_qk mg[b,q]*prodc[b,(q,k)] + mg[b,100],
  fused into the context branch -- no PE transposes anywhere.

* Embedding rows are fetched with two-stage dma_gather, spread over all
  4 SWDGE queues (independent Q7 pairs + DMA rings):
    stage 1: 4 value-range segments (25000 rows each, int16-addressable)
             gathered HBM -> SBUF scratch [128, 20, 384] bf16.
    stage 2: SBUF-source transpose-mode gather (xbar) realigns scratch
             slots to batch order and directly emits the [e, token] lhsT
             layout [128, 3, 512] per quarter.  No DRAM scratch bounce.
  4 tiny warmup gathers (one per queue) absorb the Q7 library-load /
  first-call overhead during the const-DMA window, and keep the 8-lane
  DMASW semaphore rotation aligned with the queue pattern (sems are
  queue-locked; queue = lane mod 4 throughout).

* Per-core partial loss summed on host (8 scalars).
"""

import sys

for _p in ("/opt/trn_rl_repo", "/opt/trn_rl_repo/concourse"):
    if _p not in sys.path:
        sys.path.insert(0, _p)

from contextlib import ExitStack

import ml_dtypes
import numpy as np

import concourse.bacc as bacc
import concourse.bass as bass
import concourse.tile as tile
from concourse import mybir
from concourse.bass_utils import run_bass_kernel_spmd

F32 = mybir.dt.float32
BF16 = mybir.dt.bfloat16
I16 = mybir.dt.int16
AF = mybir.ActivationFunctionType
AX = mybir.AxisListType
OP = mybir.AluOpType

N_CORES = 8
B = 16384
BC = B // N_CORES          # 2048 batch items per core
NB = BC // 128             # 16 chunks of 128 batch rows
NQ = 4                     # gather queues / batch quarters
BQ = BC // NQ              # 512 tokens per stage-2 gather
V = 100000
EMB = 300
EPAD = 384                 # padded embedding row (col 300 = 1.0, rest 0)
H = 100                    # h1 = h2 = prod dims
MH = H + 1                 # homogeneous size
R = 3                      # tv-curve basis rank
NPR = H * R                # 300 contracted cols (context branch)
NQR = MH * R               # 303 contracted cols (target branch, Gram-folded)
SEG = 25000                # value-range segment size (int16-addressable)
CAP = 640                  # fixed per-segment slot capacity (multiple of 128)
NSLOT = NQ * CAP           # 2560 scratch slots per branch
NCP = 222                  # packed const cols: h2kb|h1k|h1b|vr|ghrow|labels
NIX = 2 * (NSLOT // 16) + 2 * (BC // 16)   # packed idx cols (s1 t/c, s2 t/c)


def _wrap16(v):
    """int16 index array -> dma_gather SBUF layout [128, len//16]."""
    v = np.asarray(v, dtype=np.int16)
    a = v.reshape(-1, 16).T          # [16, len/16]; slot j at [j%16, j//16]
    return np.tile(a, (8, 1))        # replicate across the 8 q7 cores


def _prep_indices(idx):
    """Partition a branch's indices into 4 value-range segments.

    Returns (seg_idx [128, NSLOT//16], slot_idx [128, BC//16]) int16:
    seg_idx holds per-segment local row offsets (batch order, pad 0);
    slot_idx maps batch position b -> scratch slot of its gathered row.
    """
    idx = np.asarray(idx).astype(np.int64)
    assert idx.shape == (BC,)
    seg = idx // SEG
    local = (idx - seg * SEG).astype(np.int16)
    seg_cols = []
    slot = np.empty(BC, dtype=np.int64)
    for j in range(NQ):
        pos = np.nonzero(seg == j)[0]
        n = len(pos)
        assert n <= CAP, f"segment {j} overflow: {n} > {CAP}"
        arr = np.zeros(CAP, dtype=np.int16)
        arr[:n] = local[pos]
        seg_cols.append(_wrap16(arr))
        slot[pos] = CAP * j + np.arange(n)
    s2 = [_wrap16(slot[BQ * q:BQ * (q + 1)]) for q in range(NQ)]
    return np.hstack(seg_cols), np.hstack(s2)


def _build_kernel(ctx: ExitStack, tc: "tile.TileContext", io: dict,
                  queue_of=None):
    """queue_of: emission-index -> SWDGE queue.  None = all queue 0.

    Returns the list of emitted dma_gather instruction names (emission
    order), used by the two-pass build to align queues with the
    round-robin DMASW semaphore lanes (sems are queue-locked, but the
    tile scheduler may reorder gathers, so the lane a gather lands on is
    only known post-schedule).
    """
    nc = tc.nc
    gather_names = []
    gq = (lambda i: 0) if queue_of is None else queue_of

    def dma_gather(*args, **kwargs):
        kwargs["queue_num"] = gq(len(gather_names))
        inst = nc.gpsimd.dma_gather(*args, **kwargs)
        gather_names.append(inst.ins.name)
        return inst

    cpool = ctx.enter_context(tc.tile_pool(name="const", bufs=1))
    pmm = ctx.enter_context(tc.tile_pool(name="pmm", bufs=4, space="PSUM"))
    pmisc = ctx.enter_context(tc.tile_pool(name="pmisc", bufs=3, space="PSUM"))
    wpool = ctx.enter_context(tc.tile_pool(name="work", bufs=6))
    lpool = ctx.enter_context(tc.tile_pool(name="loss", bufs=2))

    # ---- consts: <=7 HWDGE DMAs total so no DMAHW-lane sharing creates
    # false waits (8 rotation-locked lanes; 18 DMAs previously aliased
    # the gathers' idx waits onto the big wg/wr loads).
    ixp = cpool.tile([128, NIX], I16, tag="ixp")
    nc.sync.dma_start(out=ixp[:], in_=io["ixp"][:, :])
    cp = cpool.tile([128, NCP], F32, tag="cp")
    nc.sync.dma_start(out=cp[:], in_=io["cp"][:, :])
    times = cpool.tile([1, BC], F32, tag="times")
    nc.sync.dma_start(out=times[:], in_=io["times"][:, :])
    ns = NSLOT // 16
    idx1 = {"t": ixp[:, 0:ns], "c": ixp[:, ns:2 * ns]}
    idx2 = {"t": ixp[:, 2 * ns:2 * ns + BC // 16],
            "c": ixp[:, 2 * ns + BC // 16:NIX]}
    h2kb = cp[0:MH, 0:H]
    h1k = cp[0:H, 100:101]
    h1b = cp[0:H, 101:102]
    vr = cp[0:H, 102:102 + R]
    ghrow = cp[:, 105:105 + MH]
    labels = cp[:, 206:206 + NB]

    ones1 = cpool.tile([1, H], F32, tag="ones1")
    nc.vector.memset(ones1[:], 1.0)
    ones128 = cpool.tile([128, 1], F32, tag="ones128")
    nc.vector.memset(ones128[:], 1.0)
    widx = cpool.tile([128, 1], I16, tag="widx")
    nc.vector.memset(widx[:], 0)

    # hoisted num_idxs registers (one MOVE each instead of one per gather)
    r_warm = nc.gpsimd.to_reg(16)
    r_cap = nc.gpsimd.to_reg(CAP)
    r_bq = nc.gpsimd.to_reg(BQ)

    # ---- gathers -------------------------------------------------------
    # 4 warmup gathers absorb the Q7 library-load/first-call cost.
    tabs = {"t": io["ttab"], "c": io["ctab"]}
    for q in range(NQ):
        wdum = cpool.tile([128, 1, EPAD], BF16, tag=f"wdum{q}",
                          name=f"wdum{q}")
        dma_gather(wdum[:], tabs["t"][0:SEG, :], widx[:],
                   16, r_warm, EPAD)
    scratch, embT = {}, {}
    for br in ("t", "c"):
        scratch[br] = cpool.tile([128, NSLOT // 128, EPAD], BF16,
                                 tag=f"scr_{br}", name=f"scr_{br}")
        embT[br] = [cpool.tile([128, 3, BQ], BF16, tag=f"embT_{br}{q}",
                               name=f"embT_{br}{q}") for q in range(NQ)]
    nblk = CAP // 128
    def s1(br, idx):
        for j in range(NQ):
            dma_gather(
                scratch[br][:, nblk * j:nblk * (j + 1), :],
                tabs[br][SEG * j:SEG * j + SEG, :],
                idx[:, (CAP // 16) * j:(CAP // 16) * (j + 1)],
                CAP, r_cap, EPAD,
            )

    def s2(br):
        for q in range(NQ):
            dma_gather(
                embT[br][q][:], scratch[br][:, :, :],
                idx2[br][:, (BQ // 16) * q:(BQ // 16) * (q + 1)],
                BQ, r_bq, EPAD, transpose=True,
                sbuf_tokens_per_rank=128,
                sbuf_free_dim_per_rank=EPAD * 2,
            )

    s1("t", idx1["t"])
    s1("c", idx1["c"])
    s2("t")
    s2("c")

    # ---- big weight tables (scalar queue, overlap with gathers) --------
    wgr = [cpool.tile([128, NQR + NPR], BF16, tag=f"wgr{j}", name=f"wgr{j}")
           for j in range(3)]
    for j in range(3):
        nc.scalar.dma_start(out=wgr[j][:],
                            in_=io["wgr"][128 * j:128 * (j + 1), :])
    wg = [w[:, 0:NQR] for w in wgr]
    wr = [w[:, NQR:NQR + NPR] for w in wgr]

    # ---- time MLP, batched over all 2048 items ------------------------
    h1T = cpool.tile([MH, BC], F32, tag="h1T")
    nc.vector.memset(h1T[:], 1.0)
    for s in range(4):
        sl = slice(512 * s, 512 * (s + 1))
        bc = pmisc.tile([H, 512], F32, tag="pm", name=f"pbc{s}")
        nc.tensor.matmul(bc[:], ones1[:], times[0:1, sl], start=True, stop=True)
        nc.scalar.activation(h1T[0:H, sl], bc[:], AF.Tanh, bias=h1b[:],
                             scale=h1k[:])
    tvT = cpool.tile([H, BC], F32, tag="tvT")
    for s in range(4):
        sl = slice(512 * s, 512 * (s + 1))
        tvp = pmisc.tile([H, 512], F32, tag="pm", name=f"ptv{s}")
        nc.tensor.matmul(tvp[:], h2kb[:], h1T[:, sl], start=True, stop=True)
        nc.scalar.activation(tvT[:, sl], tvp[:], AF.Tanh)
    cbs = []
    for c in range(NB):
        cbp = pmisc.tile([128, R], F32, tag="pm", name=f"pcb{c}")
        nc.tensor.matmul(cbp[:], tvT[:, 128 * c:128 * (c + 1)], vr[:],
                         start=True, stop=True)
        cb = cpool.tile([128, R], BF16, tag=f"cb{c}", name=f"cb{c}")
        nc.scalar.copy(cb[:], cbp[:])
        cbs.append(cb)

    # ---- per-chunk branch contraction ---------------------------------
    def branch_prod(br, c, wt, ngrp):
        """prod[b,(g,k)] = (embT_chunk.T @ wt)[b,(g,k)] * cb[b,k], bf16."""
        qq, c2 = divmod(c, NQ)
        ncols = ngrp * R
        mp = pmm.tile([128, NQR], F32, tag="mp", name=f"mp_{br}{c}")
        for j in range(3):
            nc.tensor.matmul(
                mp[:, 0:ncols],
                embT[br][qq][:, j, 128 * c2:128 * (c2 + 1)],
                wt[j][:, 0:ncols], start=(j == 0), stop=(j == 2),
            )
        ms = wpool.tile([128, ngrp, R], BF16, tag="ms")
        nc.scalar.copy(ms[:].rearrange("p a k -> p (a k)"), mp[:, 0:ncols])
        prod = wpool.tile([128, ngrp, R], BF16, tag="prod")
        nc.vector.tensor_mul(
            prod[:], ms[:],
            cbs[c][:].unsqueeze(1).broadcast_to((128, ngrp, R)),
        )
        return prod

    # target branch: Gram-folded -> mg[b,q] (+ constant row Gh[100,:])
    mgs = []
    mgcol = cpool.tile([128, NB], F32, tag="mgcol")
    for c in range(NB):
        prod = branch_prod("t", c, wg, MH)
        tmp = wpool.tile([128, MH], F32, tag="mgtmp")
        nc.vector.reduce_sum(out=tmp[:], in_=prod[:], axis=AX.X)
        mg = cpool.tile([128, MH], F32, tag=f"mg{c}", name=f"mg{c}")
        nc.vector.tensor_add(mg[:], tmp[:], ghrow[:])
        nc.vector.tensor_copy(mgcol[:, c:c + 1], mg[:, H:MH])
        mgs.append(mg)

    # context branch + fused logits dot
    logits = cpool.tile([128, NB], F32, tag="logits")
    for c in range(NB):
        prod = branch_prod("c", c, wr, H)
        prod2 = wpool.tile([128, H, R], BF16, tag="prod2")
        nc.vector.tensor_mul(
            prod2[:], prod[:],
            mgs[c][:, 0:H].unsqueeze(2).broadcast_to((128, H, R)),
        )
        nc.vector.reduce_sum(out=logits[:, c:c + 1], in_=prod2[:], axis=AX.XY)

    # ---- batched loss tail, two halves so the first overlaps the
    # c-loop (and pulls the Exp/Ln act-table loads off the tail) --------
    lg = lpool.tile([128, NB], F32, tag="lg")
    dvec = lpool.tile([128, NB], F32, tag="dvec")
    for h in range(2):
        s = slice(8 * h, 8 * (h + 1))
        nc.vector.tensor_add(lg[:, s], logits[:, s], mgcol[:, s])
        ab = lpool.tile([128, 8], F32, tag="ab")
        nc.scalar.activation(ab[:], lg[:, s], AF.Abs)
        ex = lpool.tile([128, 8], F32, tag="ex")
        nc.scalar.activation(ex[:], ab[:], AF.Exp, scale=-1.0)
        l1p = lpool.tile([128, 8], F32, tag="l1p")
        nc.scalar.activation(l1p[:], ex[:], AF.Ln, bias=1.0)
        rl = lpool.tile([128, 8], F32, tag="rl")
        nc.scalar.activation(rl[:], lg[:, s], AF.Relu)
        sp = lpool.tile([128, 8], F32, tag="sp")
        nc.vector.tensor_add(sp[:], rl[:], l1p[:])
        ll = lpool.tile([128, 8], F32, tag="ll")
        nc.vector.tensor_mul(ll[:], lg[:, s], labels[:, s])
        nc.vector.tensor_sub(dvec[:, s], sp[:], ll[:])

    srow = cpool.tile([128, 1], F32, tag="srow")
    nc.vector.reduce_sum(out=srow[:], in_=dvec[:], axis=AX.X)
    fin = pmisc.tile([1, 1], F32, tag="pm", name="pfin")
    nc.tensor.matmul(fin[:], srow[:], ones128[:], start=True, stop=True)
    res = cpool.tile([1, 1], F32, tag="res")
    nc.scalar.copy(res[:], fin[:])
    nc.sync.dma_start(out=io["out"][:, :], in_=res[:])
    return gather_names


_PROGRAM = None


def _gather_lanes(nc, names):
    """name -> DMASW lane (bass_scheduled_proc minus the first DMASW proc)."""
    procs = {}
    for f in nc.m.functions:
        for b in f.blocks:
            for inst in b.instructions:
                if type(inst).__name__ == "InstDMAGatherAnt":
                    procs[inst.name] = inst.bass_scheduled_proc
    base = min(procs.values())
    return [procs[n] - base for n in names]


def _build_once(queue_of):
    nc = bacc.Bacc("TRN2", target_bir_lowering=False, debug=False,
                   num_devices=N_CORES, num_swdge_queues=NQ)
    io = _declare_io(nc)
    with tile.TileContext(nc) as tc:
        with ExitStack() as ctx:
            names = _build_kernel(ctx, tc, io, queue_of)
    nc.compile()
    return nc, names


def _get_program():
    global _PROGRAM
    if _PROGRAM is not None:
        return _PROGRAM
    # pass 1: all gathers on queue 0; read the scheduled DMASW lanes
    nc1, names1 = _build_once(None)
    lanes = _gather_lanes(nc1, names1)
    qmap = [lane % NQ for lane in lanes]
    # pass 2: queue = lane % NQ keeps every queue-locked lane semaphore
    # consistent no matter how the scheduler ordered the gathers
    nc2, names2 = _build_once(lambda i: qmap[i])
    lanes2 = _gather_lanes(nc2, names2)
    for q, lane in zip(qmap, lanes2):
        assert lane % NQ == q, (qmap, lanes2)
    _PROGRAM = nc2
    return nc2


def _declare_io(nc):
    io = {
        "ttab": nc.dram_tensor("ttab", [V, EPAD], BF16, kind="ExternalInput").ap(),
        "ctab": nc.dram_tensor("ctab", [V, EPAD], BF16, kind="ExternalInput").ap(),
        "wgr": nc.dram_tensor("wgr", [EPAD, NQR + NPR], BF16, kind="ExternalInput").ap(),
        "cp": nc.dram_tensor("cp", [128, NCP], F32, kind="ExternalInput").ap(),
        "times": nc.dram_tensor("times", [1, BC], F32, kind="ExternalInput").ap(),
        "ixp": nc.dram_tensor("ixp", [128, NIX], I16, kind="ExternalInput").ap(),
        "out": nc.dram_tensor("out", [1, 1], F32, kind="ExternalOutput").ap(),
    }
    return io


def _pad_table(tab):
    out = np.zeros((V, EPAD), dtype=ml_dtypes.bfloat16)
    out[:, :EMB] = np.asarray(tab).astype(ml_dtypes.bfloat16)
    out[:, EMB] = 1.0
    return out


def _tv_basis(h1_k, h1_b, h2_k, h2_b):
    """Top-R right singular basis of the tv curve (weights-only precompute)."""
    g = np.linspace(0.0, 1.0, 8193, dtype=np.float64).reshape(-1, 1)
    h1 = np.tanh(g @ np.asarray(h1_k, np.float64).reshape(1, H)
                 + np.asarray(h1_b, np.float64).reshape(H))
    tvg = np.tanh(h1 @ np.asarray(h2_k, np.float64)
                  + np.asarray(h2_b, np.float64).reshape(H))
    _, _, vt = np.linalg.svd(tvg, full_matrices=False)
    return np.ascontiguousarray(vt[:R].T)          # [100, R]


def build_in_maps(targets, contexts, times, labels, targetemb, contextemb,
                  h1_k, h1_b, h2_k, h2_b, evoke_k, evoke_b, last_k, last_b):
    ttab = _pad_table(targetemb)
    ctab = _pad_table(contextemb)
    vrb = _tv_basis(h1_k, h1_b, h2_k, h2_b)        # [100, R] float64
    evoke_pad = np.zeros((EPAD, H * H), dtype=np.float64)
    evoke_pad[:EMB, :] = np.asarray(evoke_k, np.float64)
    evoke_pad[EMB, :] = np.asarray(evoke_b, np.float64)
    # Wr[e, (p, k)] = sum_h evoke_pad[e, p*100+h] * Vr[h, k]
    wrm = (evoke_pad.reshape(EPAD * H, H) @ vrb).reshape(EPAD, H, R)
    # Gram of homogeneous last layer: Gh[p, q], p,q in [0, 101)
    lastkh = np.vstack([np.asarray(last_k, np.float64),
                        np.asarray(last_b, np.float64).reshape(1, EMB)])
    gh = lastkh @ lastkh.T                          # [101, 101]
    # WG[e, (q, k)] = sum_p Wr[e, p, k] * Gh[p, q]   (p < 100)
    wgm = np.einsum("epk,pq->eqk", wrm, gh[:H, :]).reshape(EPAD, NQR)
    wgr = np.hstack([wgm, wrm.reshape(EPAD, NPR)]).astype(ml_dtypes.bfloat16)
    cp = np.zeros((128, NCP), dtype=np.float32)
    cp[0:MH, 0:H] = np.vstack([np.asarray(h2_k),
                               np.asarray(h2_b).reshape(1, H)])
    cp[0:H, 100] = np.asarray(h1_k).reshape(H)
    cp[0:H, 101] = np.asarray(h1_b).reshape(H)
    cp[0:H, 102:102 + R] = vrb
    cp[:, 105:105 + MH] = gh[H, :]           # ghrow, replicated rows
    targets = np.asarray(targets)
    contexts = np.asarray(contexts)
    times = np.asarray(times).astype(np.float32)
    labels = np.asarray(labels).astype(np.float32)

    in_maps = []
    for k in range(N_CORES):
        sl = slice(k * BC, (k + 1) * BC)
        idx1_t, idx2_t = _prep_indices(targets[sl])
        idx1_c, idx2_c = _prep_indices(contexts[sl])
        cpk = cp.copy()
        cpk[:, 206:206 + NB] = labels[sl].reshape(NB, 128).T
        in_maps.append({
            "ttab": ttab, "ctab": ctab, "wgr": wgr, "cpk_": None,
            "cp": cpk,
            "times": times[sl].reshape(1, BC),
            "ixp": np.hstack([idx1_t, idx1_c, idx2_t, idx2_c]),
        })
        in_maps[-1].pop("cpk_")
    return in_maps


def kernel(**inputs) -> np.ndarray:
    nc = _get_program()
    in_maps = build_in_maps(**inputs)
    r = run_bass_kernel_spmd(nc, in_maps, list(range(N_CORES)))
    total = np.float64(0.0)
    for m in r.results:
        total += np.float64(m["out"][0, 0])
    return np.float32(total / B)


# revision 29
# speedup vs baseline: 1.0079x; 1.0079x over previous
"""Trainium2 Bass kernel for nn_DiffTime (embedding_lookup, 8 NeuronCores).

Computation (see reference):
    h1 = tanh(times * h1_k + h1_b)            [B, 100]
    tv = tanh(h1 @ h2_k + h2_b)               [B, 100]
    mat_x = (emb_x @ evoke_k + evoke_b)       [B, 100p, 100h]   (x in {target, context})
    mv_x = einsum('bph,bh->bp', mat_x, tv)    [B, 100]
    vect_x = mv_x @ last_k + last_b           [B, 300]
    logits = sum(vect_t * vect_c, -1)         [B]
    out = mean(softplus(logits) - logits * labels)

Strategy (data-parallel over batch, 2048 items/core, no collectives):

* tv[b,:] lies on a 1-D curve in R^100; its SVD collapses to rank R=3
  (loss rel err ~8e-7 incl. bf16).  The kernel contracts emb with
  Wr[e,(p,k)] = sum_h evoke[e,p*100+h]*Vr[h,k] and forms
  mv[b,p] = sum_k matU[b,p,k]*cb[b,k] on the DVE (stride-0 broadcast of
  cb, grouped reduce), in bf16.

* Gram fold: for the target branch the kernel uses
  WG[e,(q,k)] = sum_p Wr[e,(p,k)] * Gh[p,q], Gh = lastkh @ lastkh.T
  (host precompute, weights only), so the branch directly yields
  mg[b,q] = sum_p mvt_h[b,p] Gh[p,q] (after adding the constant row
  Gh[100,:]).  logits[b] = sum_qk mg[b,q]*prodc[b,(q,k)] + mg[b,100],
  fused into the context branch -- no PE transposes anywhere.

* Embedding rows are fetched with two-stage dma_gather, spread over all
  4 SWDGE queues (independent Q7 pairs + DMA rings):
    stage 1: 4 value-range segments (25000 rows each, int16-addressable)
             gathered HBM -> SBUF scratch [128, 20, 384] bf16.
    stage 2: SBUF-source transpose-mode gather (xbar) realigns scratch
             slots to batch order and directly emits the [e, token] lhsT
             layout [128, 3, 512] per quarter.  No DRAM scratch bounce.
  4 tiny warmup gathers (one per queue) absorb the Q7 library-load /
  first-call overhead during the const-DMA window, and keep the 8-lane
  DMASW semaphore rotation aligned with the queue pattern (sems are
  queue-locked; queue = lane mod 4 throughout).

* Per-core partial loss summed on host (8 scalars).
"""

import sys

for _p in ("/opt/trn_rl_repo", "/opt/trn_rl_repo/concourse"):
    if _p not in sys.path:
        sys.path.insert(0, _p)

from contextlib import ExitStack

import ml_dtypes
import numpy as np

import concourse.bacc as bacc
import concourse.bass as bass
import concourse.tile as tile
from concourse import mybir
from concourse.bass_utils import run_bass_kernel_spmd

F32 = mybir.dt.float32
BF16 = mybir.dt.bfloat16
I16 = mybir.dt.int16
AF = mybir.ActivationFunctionType
AX = mybir.AxisListType
OP = mybir.AluOpType

N_CORES = 8
B = 16384
BC = B // N_CORES          # 2048 batch items per core
NB = BC // 128             # 16 chunks of 128 batch rows
NQ = 4                     # gather queues / batch quarters
BQ = BC // NQ              # 512 tokens per stage-2 gather
V = 100000
EMB = 300
EPAD = 384                 # padded embedding row (col 300 = 1.0, rest 0)
H = 100                    # h1 = h2 = prod dims
MH = H + 1                 # homogeneous size
R = 3                      # tv-curve basis rank
NPR = H * R                # 300 contracted cols (context branch)
NQR = MH * R               # 303 contracted cols (target branch, Gram-folded)
SEG = 25000                # value-range segment size (int16-addressable)
CAP = 640                  # fixed per-segment slot capacity (multiple of 128)
NSLOT = NQ * CAP           # 2560 scratch slots per branch
NCP = 222                  # packed const cols: h2kb|h1k|h1b|vr|ghrow|labels
NIX = 2 * (NSLOT // 16) + 2 * (BC // 16)   # packed idx cols (s1 t/c, s2 t/c)


def _wrap16(v):
    """int16 index array -> dma_gather SBUF layout [128, len//16]."""
    v = np.asarray(v, dtype=np.int16)
    a = v.reshape(-1, 16).T          # [16, len/16]; slot j at [j%16, j//16]
    return np.tile(a, (8, 1))        # replicate across the 8 q7 cores


def _prep_indices(idx):
    """Partition a branch's indices into 4 value-range segments.

    Returns (seg_idx [128, NSLOT//16], slot_idx [128, BC//16]) int16:
    seg_idx holds per-segment local row offsets (batch order, pad 0);
    slot_idx maps batch position b -> scratch slot of its gathered row.
    """
    idx = np.asarray(idx).astype(np.int64)
    assert idx.shape == (BC,)
    seg = idx // SEG
    local = (idx - seg * SEG).astype(np.int16)
    seg_cols = []
    slot = np.empty(BC, dtype=np.int64)
    for j in range(NQ):
        pos = np.nonzero(seg == j)[0]
        n = len(pos)
        assert n <= CAP, f"segment {j} overflow: {n} > {CAP}"
        arr = np.zeros(CAP, dtype=np.int16)
        arr[:n] = local[pos]
        seg_cols.append(_wrap16(arr))
        slot[pos] = CAP * j + np.arange(n)
    s2 = [_wrap16(slot[BQ * q:BQ * (q + 1)]) for q in range(NQ)]
    return np.hstack(seg_cols), np.hstack(s2)


def _build_kernel(ctx: ExitStack, tc: "tile.TileContext", io: dict,
                  queue_of=None):
    """queue_of: emission-index -> SWDGE queue.  None = all queue 0.

    Returns the list of emitted dma_gather instruction names (emission
    order), used by the two-pass build to align queues with the
    round-robin DMASW semaphore lanes (sems are queue-locked, but the
    tile scheduler may reorder gathers, so the lane a gather lands on is
    only known post-schedule).
    """
    nc = tc.nc
    gather_names = []
    gq = (lambda i: 0) if queue_of is None else queue_of

    def dma_gather(*args, **kwargs):
        kwargs["queue_num"] = gq(len(gather_names))
        inst = nc.gpsimd.dma_gather(*args, **kwargs)
        gather_names.append(inst.ins.name)
        return inst

    cpool = ctx.enter_context(tc.tile_pool(name="const", bufs=1))
    pmm = ctx.enter_context(tc.tile_pool(name="pmm", bufs=3, space="PSUM"))
    pmisc = ctx.enter_context(tc.tile_pool(name="pmisc", bufs=3, space="PSUM"))
    wpool = ctx.enter_context(tc.tile_pool(name="work", bufs=4))
    lpool = ctx.enter_context(tc.tile_pool(name="loss", bufs=2))

    # ---- consts: <=7 HWDGE DMAs total so no DMAHW-lane sharing creates
    # false waits (8 rotation-locked lanes; 18 DMAs previously aliased
    # the gathers' idx waits onto the big wg/wr loads).
    times = cpool.tile([1, BC], F32, tag="times")
    nc.sync.dma_start(out=times[:], in_=io["times"][:, :])
    cp = cpool.tile([128, NCP], F32, tag="cp")
    nc.sync.dma_start(out=cp[:], in_=io["cp"][:, :])
    ixp = cpool.tile([128, NIX], I16, tag="ixp")
    nc.sync.dma_start(out=ixp[:], in_=io["ixp"][:, :])
    ns = NSLOT // 16
    idx1 = {"t": ixp[:, 0:ns], "c": ixp[:, ns:2 * ns]}
    idx2 = {"t": ixp[:, 2 * ns:2 * ns + BC // 16],
            "c": ixp[:, 2 * ns + BC // 16:NIX]}
    h2kb = cp[0:MH, 0:H]
    h1k = cp[0:H, 100:101]
    h1b = cp[0:H, 101:102]
    vr = cp[0:H, 102:102 + R]
    ghrow = cp[:, 105:105 + MH]
    labels = cp[:, 206:206 + NB]

    ones1 = cpool.tile([1, H], F32, tag="ones1")
    nc.vector.memset(ones1[:], 1.0)
    ones128 = cpool.tile([128, 1], F32, tag="ones128")
    nc.vector.memset(ones128[:], 1.0)
    widx = cpool.tile([128, 1], I16, tag="widx")
    nc.vector.memset(widx[:], 0)

    # hoisted num_idxs registers (one MOVE each instead of one per gather)
    r_warm = nc.gpsimd.to_reg(16)
    r_cap = nc.gpsimd.to_reg(CAP)
    r_bq = nc.gpsimd.to_reg(BQ)

    # ---- gathers -------------------------------------------------------
    # 4 warmup gathers absorb the Q7 library-load/first-call cost.
    tabs = {"t": io["ttab"], "c": io["ctab"]}
    for q in range(NQ):
        wdum = cpool.tile([128, 1, EPAD], BF16, tag=f"wdum{q}",
                          name=f"wdum{q}")
        dma_gather(wdum[:], tabs["t"][0:SEG, :], widx[:],
                   16, r_warm, EPAD)
    scratch, embT = {}, {}
    for br in ("t", "c"):
        scratch[br] = cpool.tile([128, NSLOT // 128, EPAD], BF16,
                                 tag=f"scr_{br}", name=f"scr_{br}")
        embT[br] = [cpool.tile([128, 3, BQ], BF16, tag=f"embT_{br}{q}",
                               name=f"embT_{br}{q}") for q in range(NQ)]
    nblk = CAP // 128
    def s1(br, idx):
        for j in range(NQ):
            dma_gather(
                scratch[br][:, nblk * j:nblk * (j + 1), :],
                tabs[br][SEG * j:SEG * j + SEG, :],
                idx[:, (CAP // 16) * j:(CAP // 16) * (j + 1)],
                CAP, r_cap, EPAD,
            )

    def s2(br):
        for q in range(NQ):
            dma_gather(
                embT[br][q][:], scratch[br][:, :, :],
                idx2[br][:, (BQ // 16) * q:(BQ // 16) * (q + 1)],
                BQ, r_bq, EPAD, transpose=True,
                sbuf_tokens_per_rank=128,
                sbuf_free_dim_per_rank=EPAD * 2,
            )

    s1("t", idx1["t"])
    s1("c", idx1["c"])
    s2("t")
    s2("c")

    # ---- big weight tables (scalar queue, overlap with gathers) --------
    wgr = [cpool.tile([128, NQR + NPR], BF16, tag=f"wgr{j}", name=f"wgr{j}")
           for j in range(3)]
    for j in range(3):
        nc.scalar.dma_start(out=wgr[j][:],
                            in_=io["wgr"][128 * j:128 * (j + 1), :])
    wg = [w[:, 0:NQR] for w in wgr]
    wr = [w[:, NQR:NQR + NPR] for w in wgr]

    # ---- time MLP, batched over all 2048 items ------------------------
    h1T = cpool.tile([MH, BC], F32, tag="h1T")
    nc.vector.memset(h1T[:], 1.0)
    for s in range(4):
        sl = slice(512 * s, 512 * (s + 1))
        bc = pmisc.tile([H, 512], F32, tag="pm", name=f"pbc{s}")
        nc.tensor.matmul(bc[:], ones1[:], times[0:1, sl], start=True, stop=True)
        nc.scalar.activation(h1T[0:H, sl], bc[:], AF.Tanh, bias=h1b[:],
                             scale=h1k[:])
    tvT = cpool.tile([H, BC], F32, tag="tvT")
    for s in range(4):
        sl = slice(512 * s, 512 * (s + 1))
        tvp = pmisc.tile([H, 512], F32, tag="pm", name=f"ptv{s}")
        nc.tensor.matmul(tvp[:], h2kb[:], h1T[:, sl], start=True, stop=True)
        nc.scalar.activation(tvT[:, sl], tvp[:], AF.Tanh)
    cbs = []
    for c in range(NB):
        cbp = pmisc.tile([128, R], F32, tag="pm", name=f"pcb{c}")
        nc.tensor.matmul(cbp[:], tvT[:, 128 * c:128 * (c + 1)], vr[:],
                         start=True, stop=True)
        cb = cpool.tile([128, R], BF16, tag=f"cb{c}", name=f"cb{c}")
        nc.scalar.copy(cb[:], cbp[:])
        cbs.append(cb)

    # ---- per-chunk branch contraction ---------------------------------
    def branch_prod(br, c, wt, ngrp):
        """prod[b,(g,k)] = (embT_chunk.T @ wt)[b,(g,k)] * cb[b,k], bf16."""
        qq, c2 = divmod(c, NQ)
        ncols = ngrp * R
        mp = pmm.tile([128, NQR], F32, tag="mp", name=f"mp_{br}{c}")
        for j in range(3):
            nc.tensor.matmul(
                mp[:, 0:ncols],
                embT[br][qq][:, j, 128 * c2:128 * (c2 + 1)],
                wt[j][:, 0:ncols], start=(j == 0), stop=(j == 2),
            )
        ms = wpool.tile([128, ngrp, R], BF16, tag="ms")
        nc.scalar.copy(ms[:].rearrange("p a k -> p (a k)"), mp[:, 0:ncols])
        prod = wpool.tile([128, ngrp, R], BF16, tag="prod")
        nc.vector.tensor_mul(
            prod[:], ms[:],
            cbs[c][:].unsqueeze(1).broadcast_to((128, ngrp, R)),
        )
        return prod

    # target branch: Gram-folded -> mg[b,q] (+ constant row Gh[100,:])
    mgs = []
    mgcol = cpool.tile([128, NB], F32, tag="mgcol")
    for c in range(NB):
        prod = branch_prod("t", c, wg, MH)
        tmp = wpool.tile([128, MH], F32, tag="mgtmp")
        nc.vector.reduce_sum(out=tmp[:], in_=prod[:], axis=AX.X)
        mg = cpool.tile([128, MH], F32, tag=f"mg{c}", name=f"mg{c}")
        nc.vector.tensor_add(mg[:], tmp[:], ghrow[:])
        nc.vector.tensor_copy(mgcol[:, c:c + 1], mg[:, H:MH])
        mgs.append(mg)

    # context branch + fused logits dot
    logits = cpool.tile([128, NB], F32, tag="logits")
    for c in range(NB):
        prod = branch_prod("c", c, wr, H)
        prod2 = wpool.tile([128, H, R], BF16, tag="prod2")
        nc.vector.tensor_mul(
            prod2[:], prod[:],
            mgs[c][:, 0:H].unsqueeze(2).broadcast_to((128, H, R)),
        )
        nc.vector.reduce_sum(out=logits[:, c:c + 1], in_=prod2[:], axis=AX.XY)

    # ---- batched loss tail, two halves so the first overlaps the
    # c-loop (and pulls the Exp/Ln act-table loads off the tail) --------
    lg = lpool.tile([128, NB], F32, tag="lg")
    dvec = lpool.tile([128, NB], F32, tag="dvec")
    for h in range(2):
        s = slice(8 * h, 8 * (h + 1))
        nc.vector.tensor_add(lg[:, s], logits[:, s], mgcol[:, s])
        ab = lpool.tile([128, 8], F32, tag="ab")
        nc.scalar.activation(ab[:], lg[:, s], AF.Abs)
        ex = lpool.tile([128, 8], F32, tag="ex")
        nc.scalar.activation(ex[:], ab[:], AF.Exp, scale=-1.0)
        l1p = lpool.tile([128, 8], F32, tag="l1p")
        nc.scalar.activation(l1p[:], ex[:], AF.Ln, bias=1.0)
        rl = lpool.tile([128, 8], F32, tag="rl")
        nc.scalar.activation(rl[:], lg[:, s], AF.Relu)
        sp = lpool.tile([128, 8], F32, tag="sp")
        nc.vector.tensor_add(sp[:], rl[:], l1p[:])
        ll = lpool.tile([128, 8], F32, tag="ll")
        nc.vector.tensor_mul(ll[:], lg[:, s], labels[:, s])
        nc.vector.tensor_sub(dvec[:, s], sp[:], ll[:])

    srow = cpool.tile([128, 1], F32, tag="srow")
    nc.vector.reduce_sum(out=srow[:], in_=dvec[:], axis=AX.X)
    fin = pmisc.tile([1, 1], F32, tag="pm", name="pfin")
    nc.tensor.matmul(fin[:], srow[:], ones128[:], start=True, stop=True)
    res = cpool.tile([1, 1], F32, tag="res")
    nc.scalar.copy(res[:], fin[:])
    nc.sync.dma_start(out=io["out"][:, :], in_=res[:])
    return gather_names


_PROGRAM = None


def _gather_lanes(nc, names):
    """name -> DMASW lane (bass_scheduled_proc minus the first DMASW proc)."""
    procs = {}
    for f in nc.m.functions:
        for b in f.blocks:
            for inst in b.instructions:
                if type(inst).__name__ == "InstDMAGatherAnt":
                    procs[inst.name] = inst.bass_scheduled_proc
    base = min(procs.values())
    return [procs[n] - base for n in names]


def _build_once(queue_of):
    nc = bacc.Bacc("TRN2", target_bir_lowering=False, debug=False,
                   num_devices=N_CORES, num_swdge_queues=NQ)
    io = _declare_io(nc)
    with tile.TileContext(nc) as tc:
        with ExitStack() as ctx:
            names = _build_kernel(ctx, tc, io, queue_of)
    nc.compile()
    return nc, names


def _get_program():
    global _PROGRAM
    if _PROGRAM is not None:
        return _PROGRAM
    # pass 1: all gathers on queue 0; read the scheduled DMASW lanes
    nc1, names1 = _build_once(None)
    lanes = _gather_lanes(nc1, names1)
    qmap = [lane % NQ for lane in lanes]
    # pass 2: queue = lane % NQ keeps every queue-locked lane semaphore
    # consistent no matter how the scheduler ordered the gathers
    nc2, names2 = _build_once(lambda i: qmap[i])
    lanes2 = _gather_lanes(nc2, names2)
    for q, lane in zip(qmap, lanes2):
        assert lane % NQ == q, (qmap, lanes2)
    _PROGRAM = nc2
    return nc2


def _declare_io(nc):
    io = {
        "ttab": nc.dram_tensor("ttab", [V, EPAD], BF16, kind="ExternalInput").ap(),
        "ctab": nc.dram_tensor("ctab", [V, EPAD], BF16, kind="ExternalInput").ap(),
        "wgr": nc.dram_tensor("wgr", [EPAD, NQR + NPR], BF16, kind="ExternalInput").ap(),
        "cp": nc.dram_tensor("cp", [128, NCP], F32, kind="ExternalInput").ap(),
        "times": nc.dram_tensor("times", [1, BC], F32, kind="ExternalInput").ap(),
        "ixp": nc.dram_tensor("ixp", [128, NIX], I16, kind="ExternalInput").ap(),
        "out": nc.dram_tensor("out", [1, 1], F32, kind="ExternalOutput").ap(),
    }
    return io


def _pad_table(tab):
    out = np.zeros((V, EPAD), dtype=ml_dtypes.bfloat16)
    out[:, :EMB] = np.asarray(tab).astype(ml_dtypes.bfloat16)
    out[:, EMB] = 1.0
    return out


def _tv_basis(h1_k, h1_b, h2_k, h2_b):
    """Top-R right singular basis of the tv curve (weights-only precompute)."""
    g = np.linspace(0.0, 1.0, 8193, dtype=np.float64).reshape(-1, 1)
    h1 = np.tanh(g @ np.asarray(h1_k, np.float64).reshape(1, H)
                 + np.asarray(h1_b, np.float64).reshape(H))
    tvg = np.tanh(h1 @ np.asarray(h2_k, np.float64)
                  + np.asarray(h2_b, np.float64).reshape(H))
    _, _, vt = np.linalg.svd(tvg, full_matrices=False)
    return np.ascontiguousarray(vt[:R].T)          # [100, R]


def build_in_maps(targets, contexts, times, labels, targetemb, contextemb,
                  h1_k, h1_b, h2_k, h2_b, evoke_k, evoke_b, last_k, last_b):
    ttab = _pad_table(targetemb)
    ctab = _pad_table(contextemb)
    vrb = _tv_basis(h1_k, h1_b, h2_k, h2_b)        # [100, R] float64
    evoke_pad = np.zeros((EPAD, H * H), dtype=np.float64)
    evoke_pad[:EMB, :] = np.asarray(evoke_k, np.float64)
    evoke_pad[EMB, :] = np.asarray(evoke_b, np.float64)
    # Wr[e, (p, k)] = sum_h evoke_pad[e, p*100+h] * Vr[h, k]
    wrm = (evoke_pad.reshape(EPAD * H, H) @ vrb).reshape(EPAD, H, R)
    # Gram of homogeneous last layer: Gh[p, q], p,q in [0, 101)
    lastkh = np.vstack([np.asarray(last_k, np.float64),
                        np.asarray(last_b, np.float64).reshape(1, EMB)])
    gh = lastkh @ lastkh.T                          # [101, 101]
    # WG[e, (q, k)] = sum_p Wr[e, p, k] * Gh[p, q]   (p < 100)
    wgm = np.einsum("epk,pq->eqk", wrm, gh[:H, :]).reshape(EPAD, NQR)
    wgr = np.hstack([wgm, wrm.reshape(EPAD, NPR)]).astype(ml_dtypes.bfloat16)
    cp = np.zeros((128, NCP), dtype=np.float32)
    cp[0:MH, 0:H] = np.vstack([np.asarray(h2_k),
                               np.asarray(h2_b).reshape(1, H)])
    cp[0:H, 100] = np.asarray(h1_k).reshape(H)
    cp[0:H, 101] = np.asarray(h1_b).reshape(H)
    cp[0:H, 102:102 + R] = vrb
    cp[:, 105:105 + MH] = gh[H, :]           # ghrow, replicated rows
    targets = np.asarray(targets)
    contexts = np.asarray(contexts)
    times = np.asarray(times).astype(np.float32)
    labels = np.asarray(labels).astype(np.float32)

    in_maps = []
    for k in range(N_CORES):
        sl = slice(k * BC, (k + 1) * BC)
        idx1_t, idx2_t = _prep_indices(targets[sl])
        idx1_c, idx2_c = _prep_indices(contexts[sl])
        cpk = cp.copy()
        cpk[:, 206:206 + NB] = labels[sl].reshape(NB, 128).T
        in_maps.append({
            "ttab": ttab, "ctab": ctab, "wgr": wgr, "cpk_": None,
            "cp": cpk,
            "times": times[sl].reshape(1, BC),
            "ixp": np.hstack([idx1_t, idx1_c, idx2_t, idx2_c]),
        })
        in_maps[-1].pop("cpk_")
    return in_maps


def kernel(**inputs) -> np.ndarray:
    nc = _get_program()
    in_maps = build_in_maps(**inputs)
    r = run_bass_kernel_spmd(nc, in_maps, list(range(N_CORES)))
    total = np.float64(0.0)
    for m in r.results:
        total += np.float64(m["out"][0, 0])
    return np.float32(total / B)
